# revision 1
# baseline (speedup 1.0000x reference)
"""MultiHeadLinearAttention Trainium2 kernel (8 NeuronCores, SPMD).

Sharding: core c handles batch b = c//2, head-group g = c%2 (4 of 8 heads,
i.e. feature slice F = [256g, 256g+256) of the 512 projection features).
Each core computes k/v/q projections restricted to its head-group, the
per-head linear-attention state over the full 8192-token sequence, and a
partial output  attn_F @ out_w[:, F].T.  The host sums the two partials per
batch and adds out_b.  No cross-core collectives are needed.

Math per head h (matches the fp32 jax reference):
  proj(x)  = silu(x@w1.T + b1) * (x@w2.T + b2)
  phi(x)   = elu(x) + 1 = max(x+1, exp(min(x, 0)))
  kv[d,e]  = sum_s phi_k[s,d] v[s,e]        (64x64 per head)
  ksum[d]  = sum_s phi_k[s,d]
  attn[s,e]= (sum_d phi_q[s,d] kv[d,e]) / (sum_d phi_q[s,d] ksum[d])
  out      = attn @ out_w.T + out_b
(The reference's +1e-6 in the denominator is negligible: denominators are
O(1e5) here.)

Matmuls run in float32r (full PE rate at free-dim>=256, ~1e-4 rel err).
"""
import sys
sys.path.insert(0, '/opt/trn_rl_repo')

import numpy as np
import concourse.bass as bass
import concourse.mybir as mybir
import concourse.tile as tile
from concourse.bass import ts, ds
from concourse.bass_utils import run_bass_kernel_spmd

F32 = mybir.dt.float32
F32R = mybir.dt.float32r
AF = mybir.ActivationFunctionType
OP = mybir.AluOpType

B, S, D = 4, 8192, 512
NH, DK = 8, 64
FG = 256            # features per head-group (4 heads)
P = 128
CHUNK = 512         # tokens per streamed chunk
NCHUNK = S // CHUNK         # 16
SUBT = CHUNK // P           # 4 subtiles of 128 tokens per chunk


def _split_waits(nc, limit=1):
    """walrus here rejects >1 embedded sync-wait per instruction; move extras
    onto same-engine NoOps immediately before (program order preserves
    semantics)."""
    uid = 0
    for f in nc.m.functions:
        for blk in f.blocks:
            new = []
            for ins in blk.instructions:
                si = ins.sync_info
                if si is not None and si.on_wait is not None and len(si.on_wait) > limit:
                    waits = list(si.on_wait)
                    head, keep = waits[:-limit], waits[-limit:]
                    for w in head:
                        nop = mybir.InstNoOp(
                            name=f"wsplit_{uid}", ins=[], outs=[],
                            sync_info=mybir.SyncInfo(on_wait=[w], on_update=[]))
                        uid += 1
                        nop.engine = ins.engine
                        new.append(nop)
                    ins.sync_info = mybir.SyncInfo(
                        on_wait=keep, on_update=list(si.on_update or []))
                new.append(ins)
            blk.instructions = new


def build_nc(repeats=1):
    nc = bass.Bass()

    # --- DRAM I/O (per-core data supplied via in_maps) ---
    xkT = nc.dram_tensor("xkT", [D, S], F32R, kind="ExternalInput")
    xvT = nc.dram_tensor("xvT", [D, S], F32R, kind="ExternalInput")
    xqT = nc.dram_tensor("xqT", [D, S], F32R, kind="ExternalInput")
    wk12T = nc.dram_tensor("wk12T", [D, 2 * FG], F32R, kind="ExternalInput")
    wv12T = nc.dram_tensor("wv12T", [D, 2 * FG], F32R, kind="ExternalInput")
    wq1T = nc.dram_tensor("wq1T", [D, FG], F32R, kind="ExternalInput")
    wq2T = nc.dram_tensor("wq2T", [D, FG], F32R, kind="ExternalInput")
    bk12p = nc.dram_tensor("bk12p", [P, 2 * FG], F32R, kind="ExternalInput")
    bv12p = nc.dram_tensor("bv12p", [P, 2 * FG], F32R, kind="ExternalInput")
    bq1 = nc.dram_tensor("bq1", [P, 2], F32, kind="ExternalInput")
    bq2 = nc.dram_tensor("bq2", [P, 2], F32, kind="ExternalInput")
    woT = nc.dram_tensor("woT", [FG, D], F32R, kind="ExternalInput")
    e0 = nc.dram_tensor("e0", [P, P], F32R, kind="ExternalInput")      # row0=1
    ones_col = nc.dram_tensor("ones_col", [P, 2], F32R, kind="ExternalInput")
    sel = nc.dram_tensor("sel", [P, P], F32R, kind="ExternalInput")
    rcp_init = nc.dram_tensor("rcp_init", [P, 4 * CHUNK], F32R, kind="ExternalInput")
    bdz = nc.dram_tensor("bdz", [P, 2 * P], F32R, kind="ExternalInput")
    dkz = nc.dram_tensor("dkz", [P, 4], F32R, kind="ExternalInput")
    out = nc.dram_tensor("out", [S, D], F32, kind="ExternalOutput")

    xkT_r = xkT.rearrange("(ko p) t -> p ko t", p=P)   # [128, 4, 8192]
    xvT_r = xvT.rearrange("(ko p) t -> p ko t", p=P)
    xqT_r = xqT.rearrange("(ko p) t -> p ko t", p=P)
    wk12T_r = wk12T.rearrange("(ko p) o -> p ko o", p=P)   # [128, 4, 512]
    wv12T_r = wv12T.rearrange("(ko p) o -> p ko o", p=P)
    wq1T_r = wq1T.rearrange("(ko p) o -> p ko o", p=P)     # [128, 4, 256]
    wq2T_r = wq2T.rearrange("(ko p) o -> p ko o", p=P)
    woT_r = woT.rearrange("(ko p) o -> p ko o", p=P)       # [128, 2, 512]
    out_r = out.rearrange("(n p) f -> n p f", p=P)         # [64, 128, 512]

    with tile.TileContext(nc) as tc:
        with tc.tile_pool(name="const", bufs=1) as cpool:
            # Resident weights / constants
            wk_sb = cpool.tile([P, 4, 2 * FG], F32R)
            wv_sb = cpool.tile([P, 4, 2 * FG], F32R)
            wq1_sb = cpool.tile([P, 4, FG], F32R)
            wq2_sb = cpool.tile([P, 4, FG], F32R)
            wo_sb = cpool.tile([P, 2, D], F32R)
            bk_sb = cpool.tile([P, 2 * FG], F32R)
            bv_sb = cpool.tile([P, 2 * FG], F32R)
            bq1_sb = cpool.tile([P, 2], F32)
            bq2_sb = cpool.tile([P, 2], F32)
            e0_sb = cpool.tile([P, P], F32R)
            ones_sb = cpool.tile([P, 2], F32R)
            sel_sb = cpool.tile([P, P], F32R)
            nc.sync.dma_start(e0_sb[:], e0[:])
            nc.sync.dma_start(bk_sb[:], bk12p[:])
            nc.sync.dma_start(bv_sb[:], bv12p[:])
            nc.sync.dma_start(ones_sb[:], ones_col[:])

            # Per-head-pair numerator/denominator lhsT built at phase boundary
            bd_sb = cpool.tile([P, 2, P], F32R)      # blockdiag kv per pair
            dk_sb = cpool.tile([P, 2, 2], F32R)      # ksum columns per pair

            # reciprocal staging (double-buffered); denominators land in rows
            # 0:2 (pair0) and 32:34 (pair1); other rows stay 1.0 so the full
            # reciprocal is well-defined and the sel-matmul ignores them.
            rcp_sb = cpool.tile([P, 2, 2, CHUNK], F32R)

            for _rep in range(repeats):
              # ---------------- Phase 1: k/v projections + state ----------------
              ctx_iop2 = tc.tile_pool(name="p2_io", bufs=3)
              iop2 = ctx_iop2.__enter__()
              with tc.tile_pool(name="p1_io", bufs=2) as iop, \
                   tc.tile_pool(name="p1_sb", bufs=3) as sbp, \
                   tc.tile_pool(name="p1_ps", bufs=3, space="PSUM") as psp, \
                   tc.tile_pool(name="p1_st", bufs=1, space="PSUM") as stp:

                  state_ps = stp.tile([P, 260], F32)   # kv pair0 | kv pair1 | ksum col pairs

                  for c in range(NCHUNK):
                      kT_c = iop.tile([P, 4, CHUNK], F32R, tag="kT")
                      vT_c = iop.tile([P, 4, CHUNK], F32R, tag="vT")
                      for ki in range(4):
                          nc.sync.dma_start(kT_c[:, ki, :], xkT_r[:, ki, ds(c * CHUNK, CHUNK)])
                          if c == 0:
                              nc.sync.dma_start(wk_sb[:, ki, :], wk12T_r[:, ki, :])
                      for ki in range(4):
                          nc.sync.dma_start(vT_c[:, ki, :], xvT_r[:, ki, ds(c * CHUNK, CHUNK)])
                          if c == 0:
                              nc.sync.dma_start(wv_sb[:, ki, :], wv12T_r[:, ki, :])
                      for s in range(SUBT):
                          first = (c == 0 and s == 0)
                          last = (c == NCHUNK - 1 and s == SUBT - 1)
                          tok = ds(s * P, P)
                          psk = psp.tile([P, 2 * FG], F32, tag="proj")
                          psv = psp.tile([P, 2 * FG], F32, tag="proj")
                          nc.tensor.matmul(psk[:], e0_sb[:], bk_sb[:], start=True, stop=False)
                          for ki in range(4):
                              nc.tensor.matmul(psk[:], kT_c[:, ki, tok], wk_sb[:, ki, :],
                                               start=False, stop=(ki == 3))
                          nc.tensor.matmul(psv[:], e0_sb[:], bv_sb[:], start=True, stop=False)
                          for ki in range(4):
                              nc.tensor.matmul(psv[:], vT_c[:, ki, tok], wv_sb[:, ki, :],
                                               start=False, stop=(ki == 3))
                          # k: silu(a1) * a2 -> phi_k ; v: silu(a1) * a2
                          silk = sbp.tile([P, FG], F32, tag="silk")
                          nc.scalar.activation(silk[:], psk[:, :FG], AF.Silu)
                          kproj = sbp.tile([P, FG], F32, tag="kproj")
                          nc.vector.tensor_tensor(kproj[:], psk[:, FG:], silk[:], OP.mult)
                          mnk = sbp.tile([P, FG], F32, tag="mnk")
                          nc.vector.tensor_scalar_min(mnk[:], kproj[:], 0.0)
                          exk = sbp.tile([P, FG], F32, tag="exk")
                          nc.scalar.activation(exk[:], mnk[:], AF.Exp)
                          phik = sbp.tile([P, FG], F32R, tag="phik")
                          nc.vector.scalar_tensor_tensor(
                              phik[:], kproj[:], 1.0, exk[:], OP.add, OP.max)
                          silv = sbp.tile([P, FG], F32, tag="silv")
                          nc.scalar.activation(silv[:], psv[:, :FG], AF.Silu)
                          vproj = sbp.tile([P, FG], F32R, tag="vproj")
                          nc.vector.tensor_tensor(vproj[:], psv[:, FG:], silv[:], OP.mult)
                          # State accumulation: ONE bank holds 4 regions
                          # (kv pair0/1, ksum pair0/1). start=True clears
                          # has_written for the WHOLE bank, so only the very
                          # first state matmul may use it; the other regions'
                          # first matmuls overwrite (bits cleared) and set their
                          # own bits, after which everything accumulates.
                          for p in range(2):
                              nc.tensor.matmul(state_ps[:, ts(p, P)],
                                               phik[:, ts(p, P)], vproj[:, ts(p, P)],
                                               start=(first and p == 0), stop=last,
                                               skip_group_check=True)
                              nc.tensor.matmul(state_ps[:, ds(2 * P + 2 * p, 2)],
                                               phik[:, ts(p, P)], ones_sb[:],
                                               start=False, stop=last,
                                               skip_group_check=True)

                  # --- phase boundary: build bd (blockdiag kv) and dk (ksum cols)
                  nc.sync.dma_start(bd_sb[:], bdz.rearrange("p (g m) -> p g m", m=P))
                  nc.sync.dma_start(dk_sb[:], dkz.rearrange("p (g m) -> p g m", m=2))
                  for p in range(2):
                      nc.vector.tensor_copy(bd_sb[0:64, p, 0:64],
                                            state_ps[0:64, ds(p * P, 64)])
                      nc.vector.tensor_copy(bd_sb[64:P, p, 64:P],
                                            state_ps[64:P, ds(p * P + 64, 64)])
                      nc.vector.tensor_copy(dk_sb[0:64, p, 0:1],
                                            state_ps[0:64, ds(2 * P + 2 * p, 1)])
                      nc.vector.tensor_copy(dk_sb[64:P, p, 1:2],
                                            state_ps[64:P, ds(2 * P + 2 * p, 1)])

              # phase-2 weights load late so phase-1's first tiles win the DMA queue
              nc.sync.dma_start(wq1_sb[:], wq1T_r[:])
              nc.sync.dma_start(wq2_sb[:], wq2T_r[:])
              nc.sync.dma_start(wo_sb[:], woT_r[:])
              nc.sync.dma_start(bq1_sb[:], bq1[:])
              nc.sync.dma_start(bq2_sb[:], bq2[:])
              nc.sync.dma_start(sel_sb[:], sel[:])
              nc.sync.dma_start(rcp_sb[:], rcp_init.rearrange("p (a b t) -> p a b t", a=2, b=2))

              # ---------------- Phase 2: q projections + attention + out -------
              with tc.tile_pool(name="p2_sb", bufs=6) as sbp2, \
                   tc.tile_pool(name="p2_ps", bufs=3, space="PSUM") as psp2, \
                   tc.tile_pool(name="p2_ps_big", bufs=3, space="PSUM") as psb2, \
                   tc.tile_pool(name="p2_ps_dn", bufs=2, space="PSUM") as psd2:

                  for c in range(NCHUNK):
                      qT_c = iop2.tile([P, 4, CHUNK], F32R, tag="qT")
                      for ki in range(4):
                          nc.sync.dma_start(qT_c[:, ki, :], xqT_r[:, ki, ds(c * CHUNK, CHUNK)])
                      phiq = []
                      for m in range(2):
                          ps1 = psp2.tile([P, CHUNK], F32, tag="qproj")
                          ps2 = psp2.tile([P, CHUNK], F32, tag="qproj")
                          for ki in range(4):
                              nc.tensor.matmul(ps1[:], wq1_sb[:, ki, ts(m, P)],
                                               qT_c[:, ki, :], start=(ki == 0), stop=(ki == 3))
                          for ki in range(4):
                              nc.tensor.matmul(ps2[:], wq2_sb[:, ki, ts(m, P)],
                                               qT_c[:, ki, :], start=(ki == 0), stop=(ki == 3))
                          sil = sbp2.tile([P, CHUNK], F32, tag="sil")
                          nc.scalar.activation(sil[:], ps1[:], AF.Silu,
                                               bias=bq1_sb[:, ds(m, 1)], scale=1.0)
                          qp = sbp2.tile([P, CHUNK], F32, tag="qp")
                          nc.vector.scalar_tensor_tensor(
                              qp[:], ps2[:], bq2_sb[:, ds(m, 1)], sil[:], OP.add, OP.mult)
                          mnq = sbp2.tile([P, CHUNK], F32, tag="mnq")
                          nc.vector.tensor_scalar_min(mnq[:], qp[:], 0.0)
                          exq = sbp2.tile([P, CHUNK], F32, tag="exq")
                          nc.scalar.activation(exq[:], mnq[:], AF.Exp)
                          phm = sbp2.tile([P, CHUNK], F32R, tag="phiq")
                          nc.vector.scalar_tensor_tensor(
                              phm[:], qp[:], 1.0, exq[:], OP.add, OP.max)
                          phiq.append(phm)

                      # denominators: [2,CHUNK] per pair (fp32r MM dst must start
                      # at partition 0, so one PSUM tile per pair)
                      dn0 = psd2.tile([2, CHUNK], F32, tag="dn")
                      dn1 = psd2.tile([2, CHUNK], F32, tag="dn")
                      nc.tensor.matmul(dn0[:], dk_sb[:, 0, :], phiq[0][:],
                                       start=True, stop=True)
                      nc.tensor.matmul(dn1[:], dk_sb[:, 1, :], phiq[1][:],
                                       start=True, stop=True)
                      par = c % 2
                      with nc.allow_low_precision(reason='fp32r reciprocal feed for PE broadcast'):
                          nc.vector.reciprocal(rcp_sb[0:2, par, 0, :], dn0[:])
                          nc.vector.reciprocal(rcp_sb[0:2, par, 1, :], dn1[:])

                      outp = []
                      for p in range(2):
                          # numerator
                          nump = psb2.tile([P, CHUNK], F32, tag="mm")
                          nc.tensor.matmul(nump[:], bd_sb[:, p, :], phiq[p][:],
                                           start=True, stop=True)
                          # reciprocal rows broadcast to the pair's 128 partitions
                          rb = psb2.tile([P, CHUNK], F32, tag="mm")
                          nc.tensor.matmul(rb[:], sel_sb[:], rcp_sb[:, par, p, :],
                                           start=True, stop=True)
                          rbs = sbp2.tile([P, CHUNK], F32, tag="rbs")
                          nc.scalar.copy(rbs[:], rb[:])
                          att = sbp2.tile([P, CHUNK], F32R, tag="att")
                          nc.vector.tensor_tensor(att[:], nump[:], rbs[:], OP.mult)
                          outp.append(att)

                      for s in range(SUBT):
                          po = psb2.tile([P, D], F32, tag="mm")
                          nc.tensor.matmul(po[:], outp[0][:, ts(s, P)], wo_sb[:, 0, :],
                                           start=True, stop=False)
                          nc.tensor.matmul(po[:], outp[1][:, ts(s, P)], wo_sb[:, 1, :],
                                           start=False, stop=True)
                          ob = sbp2.tile([P, D], F32, tag="ob")
                          if s % 2 == 0:
                              nc.scalar.copy(ob[:], po[:])
                          else:
                              nc.vector.tensor_copy(ob[:], po[:])
                          nc.sync.dma_start(out_r[c * SUBT + s], ob[:])

              ctx_iop2.__exit__(None, None, None)

    _split_waits(nc)
    return nc


_NC_CACHE = None


def _get_nc():
    global _NC_CACHE
    if _NC_CACHE is None:
        _NC_CACHE = build_nc()
    return _NC_CACHE


def _prep_in_maps(inputs):
    return _build_in_maps(
        inputs["query"], inputs["key"], inputs["value"],
        inputs["q_w1"], inputs["q_w2"], inputs["k_w1"], inputs["k_w2"],
        inputs["v_w1"], inputs["v_w2"], inputs["out_w"],
        inputs["q_b1"], inputs["q_b2"], inputs["k_b1"], inputs["k_b2"],
        inputs["v_b1"], inputs["v_b2"])


def _build_in_maps(query, key, value,
                   q_w1, q_w2, k_w1, k_w2, v_w1, v_w2, out_w,
                   q_b1, q_b2, k_b1, k_b2, v_b1, v_b2):
    query = np.asarray(query, dtype=np.float32)
    key = np.asarray(key, dtype=np.float32)
    value = np.asarray(value, dtype=np.float32)

    e0 = np.zeros((P, P), np.float32); e0[0, :] = 1.0
    ones_col = np.ones((P, 2), np.float32)
    # sel[k, m]: reciprocal row j (j=0,1) -> partitions 64j..64j+63
    sel = np.zeros((P, P), np.float32)
    sel[0, 0:64] = 1.0
    sel[1, 64:128] = 1.0
    rcp_init = np.ones((P, 4 * CHUNK), np.float32)
    bdz = np.zeros((P, 2 * P), np.float32)
    dkz = np.zeros((P, 4), np.float32)

    in_maps = []
    for c in range(8):
        b, g = c // 2, c % 2
        Fs = slice(FG * g, FG * (g + 1))
        bk12p = np.zeros((P, 2 * FG), np.float32)
        bk12p[0] = np.concatenate([np.asarray(k_b1)[Fs], np.asarray(k_b2)[Fs]])
        bv12p = np.zeros((P, 2 * FG), np.float32)
        bv12p[0] = np.concatenate([np.asarray(v_b1)[Fs], np.asarray(v_b2)[Fs]])
        in_maps.append({
            "xkT": np.ascontiguousarray(key[b].T),
            "xvT": np.ascontiguousarray(value[b].T),
            "xqT": np.ascontiguousarray(query[b].T),
            "wk12T": np.ascontiguousarray(
                np.concatenate([np.asarray(k_w1)[Fs].T, np.asarray(k_w2)[Fs].T], axis=1)),
            "wv12T": np.ascontiguousarray(
                np.concatenate([np.asarray(v_w1)[Fs].T, np.asarray(v_w2)[Fs].T], axis=1)),
            "wq1T": np.ascontiguousarray(np.asarray(q_w1)[Fs].T),
            "wq2T": np.ascontiguousarray(np.asarray(q_w2)[Fs].T),
            "bk12p": bk12p,
            "bv12p": bv12p,
            "bq1": np.ascontiguousarray(np.asarray(q_b1)[Fs].reshape(2, P).T),
            "bq2": np.ascontiguousarray(np.asarray(q_b2)[Fs].reshape(2, P).T),
            "woT": np.ascontiguousarray(np.asarray(out_w)[:, Fs].T),
            "e0": e0, "ones_col": ones_col, "sel": sel, "bdz": bdz, "dkz": dkz,
            "rcp_init": rcp_init,
        })
    return in_maps


def kernel(query, key, value,
           q_w1, q_w2, k_w1, k_w2, v_w1, v_w2, out_w,
           q_b1, q_b2, k_b1, k_b2, v_b1, v_b2, out_b):
    in_maps = _build_in_maps(query, key, value,
                             q_w1, q_w2, k_w1, k_w2, v_w1, v_w2, out_w,
                             q_b1, q_b2, k_b1, k_b2, v_b1, v_b2)
    nc = _get_nc()
    res = run_bass_kernel_spmd(nc, in_maps, core_ids=list(range(8)))
    ob = np.asarray(out_b, dtype=np.float32)
    out = np.empty((B, S, D), np.float32)
    for b in range(B):
        out[b] = res.results[2 * b]["out"] + res.results[2 * b + 1]["out"] + ob
    return out

